# revision 20
# baseline (speedup 1.0000x reference)
"""Trainium2 Bass kernel for nn_Attention_61168924229643.

4-head attention over 1024 tokens, dim_head=32, with the reference's quirks:
  - l2norm over the TOKEN axis (axis=1 of (B, HW, h, d)),
  - `attn - attn.argmax(-1)` before softmax cancels inside softmax.

Sharding: B=8 batch elements -> one NeuronCore each, no collectives.

Layout: tokens on the SBUF free axis, channels on partitions.  x arrives
host-transposed/bf16-cast; attention is permutation-equivariant over tokens
and the permuted order (token 8p+t <-> column t*128+p) makes the input and
output DMAs contiguous per partition.

v3 structure per core:
  - Q^T/K^T/V from one wqkv weight tile against x^T (bf16 matmuls).
  - l2norm scales s = 10/(||q|| ||k||) folded into the block-diagonal K
    stationaries (per-partition muls); Q^T is a plain bf16 copy.
  - exp split across ScalarE (exact ACT) and VectorE (Schraudolph bit-hack:
    bf16 bits of exp(x) ~= int16(x*128*log2(e) + 127*128 - c), one
    tensor_scalar per tile) for DVE_TILES.
  - HEAD-PAIR PHASES: the full S/exp/PV pipeline runs for heads 0,1 first,
    then heads 2,3.  The h01 normalize + output projection overlap the h23
    pipeline, leaving only the h23 normalize on the critical tail.
  - PV stationaries interleave V and ones in 16-col groups
    ([V[0:16]|1|V[16:32]|1]), so O rows and denominator rows share each
    32-partition block and the reciprocal partition-remap is a single
    stream_shuffle (no SBUF-to-SBUF DMA round trip).
  - Output projection is transposed (stationary w_out halves, moving
    normalized O): y^T [c, token'] accumulates in the retired o_a banks;
    host does the final un-permute/transpose + bias add.
"""

import os
import numpy as np
import ml_dtypes
from contextlib import ExitStack

import concourse.tile as tile
from concourse import bacc, mybir
from concourse.bass_utils import run_bass_kernel_spmd

FP32 = mybir.dt.float32
BF16 = mybir.dt.bfloat16
I16 = mybir.dt.int16

HW = 1024
C = 128
HEADS = 4
DH = 32
N_CORES = 8
NT = HW // 128

# (jt, h) tiles whose exp runs on VectorE (Schraudolph) instead of ScalarE.
DVE_TILES = frozenset({(jt, h) for jt in range(2, 8) for h in (1, 3)})
EXP_A = 128 * 1.4426950408889634
EXP_C = float(os.environ.get("KEXPC", "3.5"))
N_WARM = int(os.environ.get("KWARM", "6"))
N_NORMWARM = int(os.environ.get("KNORMWARM", "14"))
N_TAILWARM = int(os.environ.get("KTAILWARM", "4"))

# stream_shuffle: within each 32-partition block, rows 0:16 <- rows 16:32
SHUF_MASK = [k + 16 if k < 16 else k for k in range(32)]


def build_kernel_body(ctx, tc, out_d, xt_d, wqkv_d, woa_d, wob_d, ones_bf_d):
    nc = tc.nc
    Exp = mybir.ActivationFunctionType.Exp
    Square = mybir.ActivationFunctionType.Square
    mult = mybir.AluOpType.mult
    add = mybir.AluOpType.add
    shr = mybir.AluOpType.logical_shift_right

    const = ctx.enter_context(tc.tile_pool(name="const", bufs=1))
    sb = ctx.enter_context(tc.tile_pool(name="sb", bufs=1))
    # PSUM banks: stp 2x2, o_a 2, o_b 2 = 8.
    stp = ctx.enter_context(tc.tile_pool(name="stp", bufs=2, space="PSUM"))
    ops = ctx.enter_context(tc.tile_pool(name="ops", bufs=1, space="PSUM"))
    rps = ctx.enter_context(tc.tile_pool(name="rps", bufs=1, space="PSUM"))

    # ---- constants: memsets on DVE (gpsimd memsets drain slowly and would
    # delay the first warm-up matmul by ~2.5us) ----
    warm = const.tile([128, 1], FP32, tag="warm")
    nc.vector.memset(warm[:], 1.0)
    warm2 = const.tile([128, 1], FP32, tag="warm2")
    nc.scalar.activation(warm2[:], warm[:], Exp)
    nc.scalar.activation(warm2[:], warm[:], Square)
    wmm_a = const.tile([128, 512], BF16, tag="wmm_a")
    nc.vector.memset(wmm_a[:], 0.25)
    ktbd = sb.tile([128, HEADS, 1024], BF16, tag="ktbd")
    # block-diagonal mask: mask4[r, h] = 1 if r//32 == h else 0; the masked
    # scale-multiply writes every ktbd row, so no big zero-memset is needed.
    mask4 = const.tile([128, 4], FP32, tag="mask4")
    nc.vector.memset(mask4[:], 0.0)
    for h in range(4):
        nc.vector.memset(mask4[32 * h:32 * (h + 1), h:h + 1], 1.0)

    # ---- input DMAs: x 4-way across the two HWDGE queues (each queue
    # moves ~21GB/s, so chunking roughly halves the x latency); wqkv split
    # K/Q/V on the gpsimd SWDGE queue in consumption order. ----
    xtb = sb.tile([128, NT * 128], BF16, tag="xtb")
    nc.sync.dma_start(xtb[:, 0:512], xt_d[:, 0:512])
    nc.scalar.dma_start(xtb[:, 512:1024], xt_d[:, 512:1024])
    wqb = sb.tile([128, 3 * C], BF16, tag="wqb")
    nc.gpsimd.dma_start(wqb[:, C:2 * C], wqkv_d[:, C:2 * C])      # K first
    nc.gpsimd.dma_start(wqb[:, 0:C], wqkv_d[:, 0:C])              # Q
    vb2 = sb.tile([128, NT, HEADS, 2 * DH], BF16, tag="vb2")
    nc.gpsimd.dma_start(vb2[:], ones_bf_d[:])
    nc.gpsimd.dma_start(wqb[:, 2 * C:3 * C], wqkv_d[:, 2 * C:3 * C])  # V
    woa = const.tile([128, C], BF16, tag="woa")
    nc.sync.dma_start(woa[:], woa_d[:])
    wob = const.tile([128, C], BF16, tag="wob")
    nc.scalar.dma_start(wob[:], wob_d[:])
    xtb_flat = xtb[:]

    # ---- PE warm-up matmuls (overlap the x DMA; HAM clock-gate food) ----
    wmm_ps = stp.tile([128, 1024], FP32, tag="st")
    for w in range(N_WARM):
        nc.tensor.matmul(
            wmm_ps[:, (w % 2) * 512:(w % 2) * 512 + 512],
            lhsT=wmm_a[:, 0:128], rhs=wmm_a[:],
            start=True, stop=True, skip_group_check=True,
        )

    # ---- K^T, Q^T in x-chunk order (start as each x chunk lands) ----
    kt_ps = stp.tile([128, 1024], FP32, tag="st")
    for ih in range(2):
        nc.tensor.matmul(
            kt_ps[:, ih * 512:(ih + 1) * 512],
            lhsT=wqb[:, C:2 * C],
            rhs=xtb_flat[:, ih * 512:(ih + 1) * 512],
            start=True, stop=True,
        )
    qt_ps = stp.tile([128, 1024], FP32, tag="st")
    for ih in range(2):
        nc.tensor.matmul(
            qt_ps[:, ih * 512:(ih + 1) * 512],
            lhsT=wqb[:, 0:C],
            rhs=xtb_flat[:, ih * 512:(ih + 1) * 512],
            start=True, stop=True,
        )

    # ---- norms ----
    nsq = sb.tile([128, 2], FP32, tag="nsq")
    qsq_scr = sb.tile([128, 1024], FP32, tag="qsq_scr")
    nc.scalar.activation(qsq_scr[:], qt_ps[:], Square, accum_out=nsq[:, 0:1])
    # Q^T plain bf16 copy on ACT (no norm dependency)
    qtb = sb.tile([128, 1024], BF16, tag="qtb")
    nc.scalar.copy(qtb[:, 0:512], qt_ps[:, 0:512])
    nc.scalar.copy(qtb[:, 512:1024], qt_ps[:, 512:1024])
    # K^T bf16 copy on DVE (feeds the scaled ktbd blocks)
    ktb = sb.tile([128, 1024], BF16, tag="ktb")
    ktb_i = nc.vector.tensor_copy(ktb[:], kt_ps[:])
    ksq_scr = sb.tile([128, 1024], FP32, tag="ksq_scr")
    nc.scalar.activation(ksq_scr[:], ktb[:], Square, scale=0.1,
                         accum_out=nsq[:, 1:2])

    # V in [token, f] orientation, parked in the o_b banks (after the norm
    # inputs so the late wqb_v DMA doesn't block the K/Q path)
    v_ps = rps.tile([128, 1024], FP32, tag="ob")
    for t in range(NT):
        nc.tensor.matmul(
            v_ps[:, t * 128:(t + 1) * 128],
            lhsT=xtb_flat[:, t * 128:(t + 1) * 128],
            rhs=wqb[:, 2 * C:3 * C],
            start=True, stop=True,
        )
    # rsq via fp32 bit-hack + 1 Newton step: [:,0]=1/||q||, [:,1]=10/||k||
    nsqc = sb.tile([128, 2], FP32, tag="nsqc")
    nc.vector.tensor_scalar_max(nsqc[:], nsq[:], 1e-26)
    nni = nsqc[:].bitcast(mybir.dt.int32)
    yi = sb.tile([128, 2], mybir.dt.int32, tag="yi")
    shr_i = nc.vector.tensor_scalar(yi[:], nni, 1, None, op0=shr)
    nc.vector.tensor_scalar(yi[:], yi[:], -1, 0x5F3759DF, op0=mult, op1=add)
    y = yi[:].bitcast(FP32)
    nh = sb.tile([128, 2], FP32, tag="nh")
    nc.vector.tensor_scalar_mul(nh[:], nsqc[:], 0.5)
    t1 = sb.tile([128, 2], FP32, tag="t1")
    nwt1 = nc.vector.tensor_mul(t1[:], y, y)
    nc.vector.tensor_mul(t1[:], t1[:], nh[:])
    nwt2 = nc.vector.tensor_scalar(t1[:], t1[:], -1.0, 1.5, op0=mult, op1=add)
    nc.vector.tensor_mul(y, y, t1[:])
    rsq = y



    # scaled block-diagonal K tiles via the masked per-partition scale
    s1 = sb.tile([128, 1], FP32, tag="s1")
    s1_i = nc.vector.tensor_scalar(s1[:], rsq[:, 0:1], rsq[:, 1:2], None,
                                   op0=mult)
    m4 = sb.tile([128, 4], FP32, tag="m4")
    m4_i = nc.vector.tensor_scalar(m4[:], mask4[:], s1[:, 0:1], None,
                                   op0=mult)

    # dummy matmuls bridge the norm-chain latency so the HAM clock-gate
    # stays warm into the S pipeline; anchors spread them across the window.
    # The o_a banks are cleared later by PV(0,0)'s start=True.
    ndum = ops.tile([128, 1024], FP32, tag="oa")
    anchors = [None, None, ktb_i, ktb_i, shr_i, shr_i, nwt1, nwt1,
               nwt2, nwt2, s1_i, s1_i, m4_i, m4_i]
    for w in range(N_NORMWARM):
        di = nc.tensor.matmul(
            ndum[:, (w % 2) * 512:(w % 2) * 512 + 256],
            lhsT=wmm_a[:, 0:128], rhs=wmm_a[:, 0:256],
            start=True, stop=True, skip_group_check=True,
        )
        a = anchors[w % len(anchors)]
        if a is not None:
            tile.add_dep_helper(di.ins, a.ins, reason="hold in norm window")

    def emit_ktbd(h):
        nc.vector.tensor_scalar(
            ktbd[:, h, :], ktb[:], m4[:, h:h + 1], None, op0=mult)
    emit_ktbd(0)
    emit_ktbd(1)

    # V scatter: vb2 slot cols [0:16]=V[:,0:16], [32:48]=V[:,16:32]
    # (ones at 16:32 and 48:64 ride in from the host background).
    v_src = v_ps[:].rearrange("p (t h s x) -> p t h s x", t=NT, h=HEADS, s=2)
    vb2_v = vb2[:].rearrange("p t h (s x) -> p t h s x", s=4)
    nc.vector.tensor_copy(vb2_v[:, :, :, 0, :], v_src[:, :, :, 0, :])
    nc.vector.tensor_copy(vb2_v[:, :, :, 2, :], v_src[:, :, :, 1, :])
    emit_ktbd(2)
    emit_ktbd(3)

    # ---- attention ----
    eb = sb.tile([128, NT, HEADS, 1024], BF16, tag="eb")
    o_a = ops.tile([128, 1024], FP32, tag="oa")  # [O0|r0 interleaved, O1|r1]
    o_b = rps.tile([128, 1024], FP32, tag="ob")  # heads 2,3

    def emit_s_exp(jt, h, st_pool_tag):
        if st_pool_tag == "oa":
            st = ops.tile([128, 1024], FP32, tag="oa", name=f"st_{jt}_{h}")
        elif st_pool_tag == "ob":
            st = rps.tile([128, 1024], FP32, tag="ob", name=f"st_{jt}_{h}")
        else:
            st = stp.tile([128, 1024], FP32, tag="st", name=f"st_{jt}_{h}")
        for ih in range(2):
            nc.tensor.matmul(
                st[:, ih * 512:(ih + 1) * 512],
                lhsT=ktbd[:, h, jt * 128:(jt + 1) * 128],
                rhs=qtb[:, ih * 512:(ih + 1) * 512],
                start=True, stop=True,
            )
        if (jt, h) in DVE_TILES:
            ebi = eb[:, jt, h, :].bitcast(I16)
            nc.vector.tensor_scalar(ebi, st[:], EXP_A, 16256.0 - EXP_C,
                                    op0=mult, op1=add)
        else:
            nc.scalar.activation(eb[:, jt, h, :], st[:], Exp)

    def emit_pv_pair(jt, heads):
        dst = o_a if heads[0] < 2 else o_b
        for ih in range(2):
            for h in heads:
                nc.tensor.matmul(
                    dst[64 * (h % 2):64 * (h % 2) + 64,
                        ih * 512:(ih + 1) * 512],
                    lhsT=vb2[:, jt, h, :],
                    rhs=eb[:, jt, h, ih * 512:(ih + 1) * 512],
                    start=(jt == 0), stop=(jt == NT - 1),
                    tile_position=(0, 64 * (h % 2)),
                    skip_group_check=True,
                )

    def normalize(o_acc, stack, tag):
        r = sb.tile([128, 1024], FP32, tag=f"r_{tag}")
        ri = nc.vector.reciprocal_approx_fast(r[:], o_acc[:])
        rs = sb.tile([128, 1024], FP32, tag=f"rs_{tag}")
        nc.vector.stream_shuffle(rs[:], r[:], SHUF_MASK)
        nc.vector.tensor_mul(stack[:], o_acc[:], rs[:])
        return ri

    stack_a = sb.tile([128, 1024], BF16, tag="stack_a")
    stack_b = sb.tile([128, 1024], BF16, tag="stack_b")

    # ---- phase A: heads 0,1 (S buffers: stp x2 + the o_b banks) ----
    tags_a = ["st", "st", "ob"]
    n = 0
    for jt in range(NT):
        for h in (0, 1):
            emit_s_exp(jt, h, tags_a[n % 3])
            n += 1
        if jt >= 2:
            emit_pv_pair(jt - 2, (0, 1))
    emit_pv_pair(NT - 2, (0, 1))
    emit_pv_pair(NT - 1, (0, 1))

    # phase-A tail (overlaps phase B): normalize h01; the projection runs at
    # the end so the retired o_a banks serve as extra phase-B S buffers.
    normalize(o_a, stack_a, "a")

    # ---- phase B: heads 2,3 (S buffers: stp x2, + the o_a banks once the
    # phase-A normalize has consumed them) ----
    tags_b = ["st"] * 16
    for i in (10, 13):
        tags_b[i] = "oa"
    n = 0
    for jt in range(NT):
        for h in (2, 3):
            emit_s_exp(jt, h, tags_b[n])
            n += 1
        if jt >= 2:
            emit_pv_pair(jt - 2, (2, 3))
    emit_pv_pair(NT - 2, (2, 3))
    emit_pv_pair(NT - 1, (2, 3))

    rbi = normalize(o_b, stack_b, "b")
    # y^T accumulates where o_a lived; proj_a starts each bank, proj_b stops.
    y_ps = ops.tile([128, 1024], FP32, tag="oa", name="y_ps")
    for ih in range(2):
        nc.tensor.matmul(
            y_ps[:, ih * 512:(ih + 1) * 512],
            lhsT=woa[:],
            rhs=stack_a[:, ih * 512:(ih + 1) * 512],
            start=True, stop=False,
            skip_group_check=True,
        )
    # keep the PE warm through the h23 normalize chain (anchored so the
    # scheduler can't hoist them into the pipeline).
    tdum = stp.tile([128, 1024], FP32, tag="st")
    for w in range(N_TAILWARM):
        di = nc.tensor.matmul(
            tdum[:, (w % 2) * 512:(w % 2) * 512 + 512],
            lhsT=wmm_a[:, 0:128], rhs=wmm_a[:],
            start=True, stop=True, skip_group_check=True,
        )
        tile.add_dep_helper(di.ins, rbi.ins, reason="hold in tail window")
    for ih in range(2):
        nc.tensor.matmul(
            y_ps[:, ih * 512:(ih + 1) * 512],
            lhsT=wob[:],
            rhs=stack_b[:, ih * 512:(ih + 1) * 512],
            start=False, stop=True,
            skip_group_check=True,
        )
    yout = sb.tile([128, 1024], BF16, tag="yout")
    nc.scalar.copy(yout[:, 0:512], y_ps[:, 0:512])
    nc.vector.tensor_copy(yout[:, 512:1024], y_ps[:, 512:1024])
    nc.sync.dma_start(out_d[0:48, :], yout[0:48, :])
    nc.scalar.dma_start(out_d[48:96, :], yout[48:96, :])
    nc.gpsimd.dma_start(out_d[96:128, :], yout[96:128, :])


def build_nc():
    nc = bacc.Bacc("TRN2", target_bir_lowering=False, debug=False,
                   num_devices=N_CORES)
    xt_d = nc.dram_tensor("xt", [128, HW], BF16, kind="ExternalInput").ap()
    wqkv_d = nc.dram_tensor("w_qkv_bf", [C, 3 * C], BF16, kind="ExternalInput").ap()
    woa_d = nc.dram_tensor("woa", [128, C], BF16, kind="ExternalInput").ap()
    wob_d = nc.dram_tensor("wob", [128, C], BF16, kind="ExternalInput").ap()
    ones_bf_d = nc.dram_tensor("ones_bf", [128, NT, HEADS, 2 * DH], BF16,
                               kind="ExternalInput").ap()
    # transposed output: y^T [c, i'] with i' = t*128 + p <-> token 8p+t
    out_d = nc.dram_tensor("out", [C, HW], BF16, kind="ExternalOutput").ap()
    with tile.TileContext(nc) as tc:
        with ExitStack() as ctx:
            build_kernel_body(ctx, tc, out_d, xt_d, wqkv_d,
                              woa_d, wob_d, ones_bf_d)
    nc.compile()
    return nc


_CACHED_NC = None


def get_nc():
    global _CACHED_NC
    if _CACHED_NC is None:
        _CACHED_NC = build_nc()
    return _CACHED_NC


def _interleave_wout_rows(w_half):
    """w_half: [64, C] (two heads' d rows).  Rows for the 16-interleaved
    stack layout: [h0 d0:16; 0; h0 d16:32; 0; h1 d0:16; 0; h1 d16:32; 0]."""
    out = np.zeros((128, C), dtype=np.float32)
    out[0:16] = w_half[0:16]
    out[32:48] = w_half[16:32]
    out[64:80] = w_half[32:48]
    out[96:112] = w_half[48:64]
    return out


def make_in_maps(x, w_qkv, w_out, b_out):
    x = np.ascontiguousarray(np.asarray(x, dtype=np.float32)).reshape(N_CORES, HW, C)
    xt = np.ascontiguousarray(
        x.reshape(N_CORES, 128, NT, C).transpose(0, 3, 2, 1).reshape(N_CORES, C, HW)
    ).astype(ml_dtypes.bfloat16)
    w_qkv_bf = np.asarray(w_qkv, dtype=np.float32).astype(ml_dtypes.bfloat16)
    w_out = np.asarray(w_out, dtype=np.float32)

    woa = _interleave_wout_rows(w_out[0:64]).astype(ml_dtypes.bfloat16)
    wob = _interleave_wout_rows(w_out[64:128]).astype(ml_dtypes.bfloat16)
    # vb2 background: ones in the 16-col denominator slots (s=1 and s=3)
    ones_bf = np.zeros((128, NT, HEADS, 2 * DH), dtype=ml_dtypes.bfloat16)
    v4 = ones_bf.reshape(128, NT, HEADS, 4, 16)
    v4[:, :, :, 1, :] = 1.0
    v4[:, :, :, 3, :] = 1.0
    return [
        {"xt": xt[i], "w_qkv_bf": w_qkv_bf, "woa": woa, "wob": wob,
         "ones_bf": ones_bf}
        for i in range(N_CORES)
    ]


def kernel(x, w_qkv, w_out, b_out, _trace=False, _trace_kwargs=None):
    nc = get_nc()
    in_maps = make_in_maps(x, w_qkv, w_out, b_out)
    res = run_bass_kernel_spmd(
        nc, in_maps, core_ids=list(range(N_CORES)),
        trace=_trace, **(_trace_kwargs or {}),
    )
    b_out_f = np.asarray(b_out, dtype=np.float32).reshape(C)
    outs = []
    for i in range(N_CORES):
        yt = np.asarray(res.results[i]["out"]).astype(np.float32)
        y = yt.reshape(C, NT, 128).transpose(2, 1, 0).reshape(HW, C)
        outs.append(y + b_out_f[None, :])
    out = np.stack(outs).reshape(8, 32, 32, 128).astype(np.float32)
    if _trace:
        kernel.last_result = res
    return out


# revision 22
# speedup vs baseline: 1.1535x; 1.1535x over previous
"""Trainium2 Bass kernel for nn_Attention_61168924229643.

4-head attention over 1024 tokens, dim_head=32, with the reference's quirks:
  - l2norm over the TOKEN axis (axis=1 of (B, HW, h, d)),
  - `attn - attn.argmax(-1)` before softmax cancels inside softmax.

Sharding: B=8 batch elements -> one NeuronCore each, no collectives.

Layout: tokens on the SBUF free axis, channels on partitions.  x arrives
host-transposed/bf16-cast; attention is permutation-equivariant over tokens
and the permuted order (token 8p+t <-> column t*128+p) makes the input and
output DMAs contiguous per partition.

v3 structure per core:
  - Q^T/K^T/V from one wqkv weight tile against x^T (bf16 matmuls).
  - l2norm scales s = 10/(||q|| ||k||) folded into the block-diagonal K
    stationaries (per-partition muls); Q^T is a plain bf16 copy.
  - exp split across ScalarE (exact ACT) and VectorE (Schraudolph bit-hack:
    bf16 bits of exp(x) ~= int16(x*128*log2(e) + 127*128 - c), one
    tensor_scalar per tile) for DVE_TILES.
  - HEAD-PAIR PHASES: the full S/exp/PV pipeline runs for heads 0,1 first,
    then heads 2,3.  The h01 normalize + output projection overlap the h23
    pipeline, leaving only the h23 normalize on the critical tail.
  - PV stationaries interleave V and ones in 16-col groups
    ([V[0:16]|1|V[16:32]|1]), so O rows and denominator rows share each
    32-partition block and the reciprocal partition-remap is a single
    stream_shuffle (no SBUF-to-SBUF DMA round trip).
  - Output projection is transposed (stationary w_out halves, moving
    normalized O): y^T [c, token'] accumulates in the retired o_a banks;
    host does the final un-permute/transpose + bias add.
"""

import os
import numpy as np
import ml_dtypes
from contextlib import ExitStack

import concourse.tile as tile
from concourse import bacc, mybir
from concourse.bass_utils import run_bass_kernel_spmd

FP32 = mybir.dt.float32
BF16 = mybir.dt.bfloat16
I16 = mybir.dt.int16

HW = 1024
C = 128
HEADS = 4
DH = 32
N_CORES = 8
NT = HW // 128

# (jt, h) tiles whose exp runs on VectorE (Schraudolph) instead of ScalarE.
DVE_TILES = frozenset({(jt, h) for jt in range(2, 8) for h in (1, 3)})
EXP_A = 128 * 1.4426950408889634
EXP_C = float(os.environ.get("KEXPC", "3.5"))
N_WARM = int(os.environ.get("KWARM", "6"))
N_NORMWARM = int(os.environ.get("KNORMWARM", "8"))
N_TAILWARM = int(os.environ.get("KTAILWARM", "4"))

# stream_shuffle: within each 32-partition block, rows 0:16 <- rows 16:32
SHUF_MASK = [k + 16 if k < 16 else k for k in range(32)]


def build_kernel_body(ctx, tc, out_d, xt_d, wqkv_d, woa_d, wob_d, ones_bf_d):
    nc = tc.nc
    Exp = mybir.ActivationFunctionType.Exp
    Square = mybir.ActivationFunctionType.Square
    mult = mybir.AluOpType.mult
    add = mybir.AluOpType.add
    shr = mybir.AluOpType.logical_shift_right

    const = ctx.enter_context(tc.tile_pool(name="const", bufs=1))
    sb = ctx.enter_context(tc.tile_pool(name="sb", bufs=1))
    # PSUM banks: stp 2x2, o_a 2, o_b 2 = 8.
    stp = ctx.enter_context(tc.tile_pool(name="stp", bufs=2, space="PSUM"))
    ops = ctx.enter_context(tc.tile_pool(name="ops", bufs=1, space="PSUM"))
    rps = ctx.enter_context(tc.tile_pool(name="rps", bufs=1, space="PSUM"))

    # ---- constants: memsets on DVE (gpsimd memsets drain slowly and would
    # delay the first warm-up matmul by ~2.5us) ----
    warm = const.tile([128, 1], FP32, tag="warm")
    nc.vector.memset(warm[:], 1.0)
    warm2 = const.tile([128, 1], FP32, tag="warm2")
    nc.scalar.activation(warm2[:], warm[:], Exp)
    nc.scalar.activation(warm2[:], warm[:], Square)
    wmm_a = const.tile([128, 512], BF16, tag="wmm_a")
    nc.vector.memset(wmm_a[:], 0.25)
    ktbd = sb.tile([128, HEADS, 1024], BF16, tag="ktbd")
    # block-diagonal mask: mask4[r, h] = 1 if r//32 == h else 0; the masked
    # scale-multiply writes every ktbd row, so no big zero-memset is needed.
    mask4 = const.tile([128, 4], FP32, tag="mask4")
    nc.vector.memset(mask4[:], 0.0)
    for h in range(4):
        nc.vector.memset(mask4[32 * h:32 * (h + 1), h:h + 1], 1.0)

    # ---- input DMAs: x 4-way across the two HWDGE queues (each queue
    # moves ~21GB/s, so chunking roughly halves the x latency); wqkv split
    # K/Q/V on the gpsimd SWDGE queue in consumption order. ----
    xtb = sb.tile([128, NT * 128], BF16, tag="xtb")
    nc.sync.dma_start(xtb[:, 0:512], xt_d[:, 0:512])
    nc.scalar.dma_start(xtb[:, 512:1024], xt_d[:, 512:1024])
    wqb = sb.tile([128, 3 * C], BF16, tag="wqb")
    nc.gpsimd.dma_start(wqb[:, C:2 * C], wqkv_d[:, C:2 * C])      # K first
    nc.gpsimd.dma_start(wqb[:, 0:C], wqkv_d[:, 0:C])              # Q
    vb2 = sb.tile([128, NT, HEADS, 2 * DH], BF16, tag="vb2")
    nc.gpsimd.dma_start(vb2[:], ones_bf_d[:])
    nc.gpsimd.dma_start(wqb[:, 2 * C:3 * C], wqkv_d[:, 2 * C:3 * C])  # V
    woa = const.tile([128, C], BF16, tag="woa")
    nc.sync.dma_start(woa[:], woa_d[:])
    wob = const.tile([128, C], BF16, tag="wob")
    nc.scalar.dma_start(wob[:], wob_d[:])
    xtb_flat = xtb[:]

    # ---- PE warm-up matmuls (overlap the x DMA; HAM clock-gate food) ----
    wmm_ps = stp.tile([128, 1024], FP32, tag="st")
    for w in range(N_WARM):
        nc.tensor.matmul(
            wmm_ps[:, (w % 2) * 512:(w % 2) * 512 + 512],
            lhsT=wmm_a[:, 0:128], rhs=wmm_a[:],
            start=True, stop=True, skip_group_check=True,
        )

    # ---- K^T, Q^T in x-chunk order (start as each x chunk lands) ----
    kt_ps = stp.tile([128, 1024], FP32, tag="st")
    for ih in range(2):
        nc.tensor.matmul(
            kt_ps[:, ih * 512:(ih + 1) * 512],
            lhsT=wqb[:, C:2 * C],
            rhs=xtb_flat[:, ih * 512:(ih + 1) * 512],
            start=True, stop=True,
        )
    qt_ps = stp.tile([128, 1024], FP32, tag="st")
    for ih in range(2):
        nc.tensor.matmul(
            qt_ps[:, ih * 512:(ih + 1) * 512],
            lhsT=wqb[:, 0:C],
            rhs=xtb_flat[:, ih * 512:(ih + 1) * 512],
            start=True, stop=True,
        )

    # ---- norms ----
    nsq = sb.tile([128, 2], FP32, tag="nsq")
    qsq_scr = sb.tile([128, 1024], FP32, tag="qsq_scr")
    nc.scalar.activation(qsq_scr[:], qt_ps[:], Square, accum_out=nsq[:, 0:1])
    # Q^T plain bf16 copy on ACT (no norm dependency)
    qtb = sb.tile([128, 1024], BF16, tag="qtb")
    nc.scalar.copy(qtb[:, 0:512], qt_ps[:, 0:512])
    nc.scalar.copy(qtb[:, 512:1024], qt_ps[:, 512:1024])
    # K^T bf16 copy on DVE (feeds the scaled ktbd blocks)
    ktb = sb.tile([128, 1024], BF16, tag="ktb")
    ktb_i = nc.vector.tensor_copy(ktb[:], kt_ps[:])
    ksq_scr = sb.tile([128, 1024], FP32, tag="ksq_scr")
    nc.scalar.activation(ksq_scr[:], ktb[:], Square, scale=0.1,
                         accum_out=nsq[:, 1:2])

    # V in [token, f] orientation, parked in the o_b banks (after the norm
    # inputs so the late wqb_v DMA doesn't block the K/Q path)
    v_ps = rps.tile([128, 1024], FP32, tag="ob")
    for t in range(NT):
        nc.tensor.matmul(
            v_ps[:, t * 128:(t + 1) * 128],
            lhsT=xtb_flat[:, t * 128:(t + 1) * 128],
            rhs=wqb[:, 2 * C:3 * C],
            start=True, stop=True,
        )
    # rsq via fp32 bit-hack + 1 Newton step: [:,0]=1/||q||, [:,1]=10/||k||
    nsqc = sb.tile([128, 2], FP32, tag="nsqc")
    nc.vector.tensor_scalar_max(nsqc[:], nsq[:], 1e-26)
    nni = nsqc[:].bitcast(mybir.dt.int32)
    yi = sb.tile([128, 2], mybir.dt.int32, tag="yi")
    shr_i = nc.vector.tensor_scalar(yi[:], nni, 1, None, op0=shr)
    nc.vector.tensor_scalar(yi[:], yi[:], -1, 0x5F3759DF, op0=mult, op1=add)
    y = yi[:].bitcast(FP32)
    nh = sb.tile([128, 2], FP32, tag="nh")
    nc.vector.tensor_scalar_mul(nh[:], nsqc[:], 0.5)
    t1 = sb.tile([128, 2], FP32, tag="t1")
    nc.vector.tensor_mul(t1[:], y, y)
    nc.vector.tensor_mul(t1[:], t1[:], nh[:])
    nwt2 = nc.vector.tensor_scalar(t1[:], t1[:], -1.0, 1.5, op0=mult, op1=add)
    nc.vector.tensor_mul(y, y, t1[:])
    rsq = y

    # dummy matmuls bridge the norm-chain latency so the HAM clock-gate
    # stays warm into the S pipeline; anchors spread them across the window.
    # The o_a banks are cleared later by PV(0,0)'s start=True.
    ndum = ops.tile([128, 1024], FP32, tag="oa")
    for w in range(N_NORMWARM):
        di = nc.tensor.matmul(
            ndum[:, (w % 2) * 512:(w % 2) * 512 + 512],
            lhsT=wmm_a[:, 0:128], rhs=wmm_a[:],
            start=True, stop=True, skip_group_check=True,
        )
        anchor = (ktb_i, ktb_i, shr_i, shr_i, shr_i, shr_i,
                  nwt2, nwt2)[w % 8]
        tile.add_dep_helper(di.ins, anchor.ins, reason="hold in norm window")

    # scaled block-diagonal K tiles via the masked per-partition scale
    s1 = sb.tile([128, 1], FP32, tag="s1")
    nc.vector.tensor_scalar(s1[:], rsq[:, 0:1], rsq[:, 1:2], None, op0=mult)
    m4 = sb.tile([128, 4], FP32, tag="m4")
    nc.vector.tensor_scalar(m4[:], mask4[:], s1[:, 0:1], None, op0=mult)

    def emit_ktbd(h):
        nc.vector.tensor_scalar(
            ktbd[:, h, :], ktb[:], m4[:, h:h + 1], None, op0=mult)
    emit_ktbd(0)
    emit_ktbd(1)

    # V scatter: vb2 slot cols [0:16]=V[:,0:16], [32:48]=V[:,16:32]
    # (ones at 16:32 and 48:64 ride in from the host background).
    v_src = v_ps[:].rearrange("p (t h s x) -> p t h s x", t=NT, h=HEADS, s=2)
    vb2_v = vb2[:].rearrange("p t h (s x) -> p t h s x", s=4)
    nc.vector.tensor_copy(vb2_v[:, :, :, 0, :], v_src[:, :, :, 0, :])
    nc.vector.tensor_copy(vb2_v[:, :, :, 2, :], v_src[:, :, :, 1, :])
    emit_ktbd(2)
    emit_ktbd(3)

    # ---- attention ----
    eb = sb.tile([128, NT, HEADS, 1024], BF16, tag="eb")
    o_a = ops.tile([128, 1024], FP32, tag="oa")  # [O0|r0 interleaved, O1|r1]
    o_b = rps.tile([128, 1024], FP32, tag="ob")  # heads 2,3

    def emit_s_exp(jt, h, st_pool_tag):
        if st_pool_tag == "oa":
            st = ops.tile([128, 1024], FP32, tag="oa", name=f"st_{jt}_{h}")
        elif st_pool_tag == "ob":
            st = rps.tile([128, 1024], FP32, tag="ob", name=f"st_{jt}_{h}")
        else:
            st = stp.tile([128, 1024], FP32, tag="st", name=f"st_{jt}_{h}")
        for ih in range(2):
            nc.tensor.matmul(
                st[:, ih * 512:(ih + 1) * 512],
                lhsT=ktbd[:, h, jt * 128:(jt + 1) * 128],
                rhs=qtb[:, ih * 512:(ih + 1) * 512],
                start=True, stop=True,
            )
        if (jt, h) in DVE_TILES:
            ebi = eb[:, jt, h, :].bitcast(I16)
            nc.vector.tensor_scalar(ebi, st[:], EXP_A, 16256.0 - EXP_C,
                                    op0=mult, op1=add)
        else:
            nc.scalar.activation(eb[:, jt, h, :], st[:], Exp)

    def emit_pv_pair(jt, heads):
        dst = o_a if heads[0] < 2 else o_b
        for ih in range(2):
            for h in heads:
                nc.tensor.matmul(
                    dst[64 * (h % 2):64 * (h % 2) + 64,
                        ih * 512:(ih + 1) * 512],
                    lhsT=vb2[:, jt, h, :],
                    rhs=eb[:, jt, h, ih * 512:(ih + 1) * 512],
                    start=(jt == 0), stop=(jt == NT - 1),
                    tile_position=(0, 64 * (h % 2)),
                    skip_group_check=True,
                )

    def normalize(o_acc, stack, tag):
        r = sb.tile([128, 1024], FP32, tag=f"r_{tag}")
        ri = nc.vector.reciprocal_approx_fast(r[:], o_acc[:])
        rs = sb.tile([128, 1024], FP32, tag=f"rs_{tag}")
        nc.vector.stream_shuffle(rs[:], r[:], SHUF_MASK)
        nc.vector.tensor_mul(stack[:], o_acc[:], rs[:])
        return ri

    stack_a = sb.tile([128, 1024], BF16, tag="stack_a")
    stack_b = sb.tile([128, 1024], BF16, tag="stack_b")

    # ---- phase A: heads 0,1 (S buffers: stp x2 + the o_b banks) ----
    tags_a = ["st", "st", "ob"]
    n = 0
    for jt in range(NT):
        for h in (0, 1):
            emit_s_exp(jt, h, tags_a[n % 3])
            n += 1
        if jt >= 2:
            emit_pv_pair(jt - 2, (0, 1))
    emit_pv_pair(NT - 2, (0, 1))
    emit_pv_pair(NT - 1, (0, 1))

    # phase-A tail (overlaps phase B): normalize h01; the projection runs at
    # the end so the retired o_a banks serve as extra phase-B S buffers.
    normalize(o_a, stack_a, "a")

    # ---- phase B: heads 2,3 (S buffers: stp x2, + the o_a banks once the
    # phase-A normalize has consumed them) ----
    tags_b = ["st"] * 16
    for i in (7, 10, 13):
        tags_b[i] = "oa"
    n = 0
    for jt in range(NT):
        for h in (2, 3):
            emit_s_exp(jt, h, tags_b[n])
            n += 1
        if jt >= 2:
            emit_pv_pair(jt - 2, (2, 3))
    emit_pv_pair(NT - 2, (2, 3))
    emit_pv_pair(NT - 1, (2, 3))

    rbi = normalize(o_b, stack_b, "b")
    # y^T accumulates where o_a lived; proj_a starts each bank, proj_b stops.
    y_ps = ops.tile([128, 1024], FP32, tag="oa", name="y_ps")
    for ih in range(2):
        nc.tensor.matmul(
            y_ps[:, ih * 512:(ih + 1) * 512],
            lhsT=woa[:],
            rhs=stack_a[:, ih * 512:(ih + 1) * 512],
            start=True, stop=False,
            skip_group_check=True,
        )
    # keep the PE warm through the h23 normalize chain (anchored so the
    # scheduler can't hoist them into the pipeline).
    tdum = stp.tile([128, 1024], FP32, tag="st")
    for w in range(N_TAILWARM):
        di = nc.tensor.matmul(
            tdum[:, (w % 2) * 512:(w % 2) * 512 + 512],
            lhsT=wmm_a[:, 0:128], rhs=wmm_a[:],
            start=True, stop=True, skip_group_check=True,
        )
        tile.add_dep_helper(di.ins, rbi.ins, reason="hold in tail window")
    for ih in range(2):
        nc.tensor.matmul(
            y_ps[:, ih * 512:(ih + 1) * 512],
            lhsT=wob[:],
            rhs=stack_b[:, ih * 512:(ih + 1) * 512],
            start=False, stop=True,
            skip_group_check=True,
        )
    yout = sb.tile([128, 1024], BF16, tag="yout")
    nc.scalar.copy(yout[:, 0:512], y_ps[:, 0:512])
    nc.vector.tensor_copy(yout[:, 512:1024], y_ps[:, 512:1024])
    nc.sync.dma_start(out_d[0:64, :], yout[0:64, :])
    nc.scalar.dma_start(out_d[64:128, :], yout[64:128, :])


def build_nc():
    nc = bacc.Bacc("TRN2", target_bir_lowering=False, debug=False,
                   num_devices=N_CORES)
    xt_d = nc.dram_tensor("xt", [128, HW], BF16, kind="ExternalInput").ap()
    wqkv_d = nc.dram_tensor("w_qkv_bf", [C, 3 * C], BF16, kind="ExternalInput").ap()
    woa_d = nc.dram_tensor("woa", [128, C], BF16, kind="ExternalInput").ap()
    wob_d = nc.dram_tensor("wob", [128, C], BF16, kind="ExternalInput").ap()
    ones_bf_d = nc.dram_tensor("ones_bf", [128, NT, HEADS, 2 * DH], BF16,
                               kind="ExternalInput").ap()
    # transposed output: y^T [c, i'] with i' = t*128 + p <-> token 8p+t
    out_d = nc.dram_tensor("out", [C, HW], BF16, kind="ExternalOutput").ap()
    with tile.TileContext(nc) as tc:
        with ExitStack() as ctx:
            build_kernel_body(ctx, tc, out_d, xt_d, wqkv_d,
                              woa_d, wob_d, ones_bf_d)
    nc.compile()
    return nc


_CACHED_NC = None


def get_nc():
    global _CACHED_NC
    if _CACHED_NC is None:
        _CACHED_NC = build_nc()
    return _CACHED_NC


def _interleave_wout_rows(w_half):
    """w_half: [64, C] (two heads' d rows).  Rows for the 16-interleaved
    stack layout: [h0 d0:16; 0; h0 d16:32; 0; h1 d0:16; 0; h1 d16:32; 0]."""
    out = np.zeros((128, C), dtype=np.float32)
    out[0:16] = w_half[0:16]
    out[32:48] = w_half[16:32]
    out[64:80] = w_half[32:48]
    out[96:112] = w_half[48:64]
    return out


def make_in_maps(x, w_qkv, w_out, b_out):
    x = np.ascontiguousarray(np.asarray(x, dtype=np.float32)).reshape(N_CORES, HW, C)
    xt = np.ascontiguousarray(
        x.reshape(N_CORES, 128, NT, C).transpose(0, 3, 2, 1).reshape(N_CORES, C, HW)
    ).astype(ml_dtypes.bfloat16)
    w_qkv_bf = np.asarray(w_qkv, dtype=np.float32).astype(ml_dtypes.bfloat16)
    w_out = np.asarray(w_out, dtype=np.float32)

    woa = _interleave_wout_rows(w_out[0:64]).astype(ml_dtypes.bfloat16)
    wob = _interleave_wout_rows(w_out[64:128]).astype(ml_dtypes.bfloat16)
    # vb2 background: ones in the 16-col denominator slots (s=1 and s=3)
    ones_bf = np.zeros((128, NT, HEADS, 2 * DH), dtype=ml_dtypes.bfloat16)
    v4 = ones_bf.reshape(128, NT, HEADS, 4, 16)
    v4[:, :, :, 1, :] = 1.0
    v4[:, :, :, 3, :] = 1.0
    return [
        {"xt": xt[i], "w_qkv_bf": w_qkv_bf, "woa": woa, "wob": wob,
         "ones_bf": ones_bf}
        for i in range(N_CORES)
    ]


def kernel(x, w_qkv, w_out, b_out, _trace=False, _trace_kwargs=None):
    nc = get_nc()
    in_maps = make_in_maps(x, w_qkv, w_out, b_out)
    res = run_bass_kernel_spmd(
        nc, in_maps, core_ids=list(range(N_CORES)),
        trace=_trace, **(_trace_kwargs or {}),
    )
    b_out_f = np.asarray(b_out, dtype=np.float32).reshape(C)
    outs = []
    for i in range(N_CORES):
        yt = np.asarray(res.results[i]["out"]).astype(np.float32)
        y = yt.reshape(C, NT, 128).transpose(2, 1, 0).reshape(HW, C)
        outs.append(y + b_out_f[None, :])
    out = np.stack(outs).reshape(8, 32, 32, 128).astype(np.float32)
    if _trace:
        kernel.last_result = res
    return out


# revision 23
# speedup vs baseline: 1.1847x; 1.0271x over previous
"""Trainium2 Bass kernel for nn_Attention_61168924229643.

4-head attention over 1024 tokens, dim_head=32, with the reference's quirks:
  - l2norm over the TOKEN axis (axis=1 of (B, HW, h, d)),
  - `attn - attn.argmax(-1)` before softmax cancels inside softmax.

Sharding: B=8 batch elements -> one NeuronCore each, no collectives.

Layout: tokens on the SBUF free axis, channels on partitions.  x arrives
host-transposed/bf16-cast; attention is permutation-equivariant over tokens
and the permuted order (token 8p+t <-> column t*128+p) makes the input and
output DMAs contiguous per partition.

v3 structure per core:
  - Q^T/K^T/V from one wqkv weight tile against x^T (bf16 matmuls).
  - l2norm scales s = 10/(||q|| ||k||) folded into the block-diagonal K
    stationaries (per-partition muls); Q^T is a plain bf16 copy.
  - exp split across ScalarE (exact ACT) and VectorE (Schraudolph bit-hack:
    bf16 bits of exp(x) ~= int16(x*128*log2(e) + 127*128 - c), one
    tensor_scalar per tile) for DVE_TILES.
  - HEAD-PAIR PHASES: the full S/exp/PV pipeline runs for heads 0,1 first,
    then heads 2,3.  The h01 normalize + output projection overlap the h23
    pipeline, leaving only the h23 normalize on the critical tail.
  - PV stationaries interleave V and ones in 16-col groups
    ([V[0:16]|1|V[16:32]|1]), so O rows and denominator rows share each
    32-partition block and the reciprocal partition-remap is a single
    stream_shuffle (no SBUF-to-SBUF DMA round trip).
  - Output projection is transposed (stationary w_out halves, moving
    normalized O): y^T [c, token'] accumulates in the retired o_a banks;
    host does the final un-permute/transpose + bias add.
"""

import os
import numpy as np
import ml_dtypes
from contextlib import ExitStack

import concourse.tile as tile
from concourse import bacc, mybir
from concourse.bass_utils import run_bass_kernel_spmd

FP32 = mybir.dt.float32
BF16 = mybir.dt.bfloat16
I16 = mybir.dt.int16

HW = 1024
C = 128
HEADS = 4
DH = 32
N_CORES = 8
NT = HW // 128

# (jt, h) tiles whose exp runs on VectorE (Schraudolph) instead of ScalarE.
DVE_TILES = frozenset({(jt, h) for jt in range(2, 8) for h in (1, 3)})
EXP_A = 128 * 1.4426950408889634
EXP_C = float(os.environ.get("KEXPC", "3.5"))
N_WARM = int(os.environ.get("KWARM", "6"))
N_NORMWARM = int(os.environ.get("KNORMWARM", "12"))
N_TAILWARM = int(os.environ.get("KTAILWARM", "8"))

# stream_shuffle: within each 32-partition block, rows 0:16 <- rows 16:32
SHUF_MASK = [k + 16 if k < 16 else k for k in range(32)]


def build_kernel_body(ctx, tc, out_d, xt_d, wqkv_d, woa_d, wob_d, ones_bf_d):
    nc = tc.nc
    Exp = mybir.ActivationFunctionType.Exp
    Square = mybir.ActivationFunctionType.Square
    mult = mybir.AluOpType.mult
    add = mybir.AluOpType.add
    shr = mybir.AluOpType.logical_shift_right

    const = ctx.enter_context(tc.tile_pool(name="const", bufs=1))
    sb = ctx.enter_context(tc.tile_pool(name="sb", bufs=1))
    # PSUM banks: stp 2x2, o_a 2, o_b 2 = 8.
    stp = ctx.enter_context(tc.tile_pool(name="stp", bufs=2, space="PSUM"))
    ops = ctx.enter_context(tc.tile_pool(name="ops", bufs=1, space="PSUM"))
    rps = ctx.enter_context(tc.tile_pool(name="rps", bufs=1, space="PSUM"))

    # ---- constants: memsets on DVE (gpsimd memsets drain slowly and would
    # delay the first warm-up matmul by ~2.5us) ----
    warm = const.tile([128, 1], FP32, tag="warm")
    nc.vector.memset(warm[:], 1.0)
    warm2 = const.tile([128, 1], FP32, tag="warm2")
    nc.scalar.activation(warm2[:], warm[:], Exp)
    nc.scalar.activation(warm2[:], warm[:], Square)
    wmm_a = const.tile([128, 512], BF16, tag="wmm_a")
    nc.vector.memset(wmm_a[:], 0.25)
    ktbd = sb.tile([128, HEADS, 1024], BF16, tag="ktbd")
    # block-diagonal mask: mask4[r, h] = 1 if r//32 == h else 0; the masked
    # scale-multiply writes every ktbd row, so no big zero-memset is needed.
    mask4 = const.tile([128, 4], FP32, tag="mask4")
    nc.vector.memset(mask4[:], 0.0)
    for h in range(4):
        nc.vector.memset(mask4[32 * h:32 * (h + 1), h:h + 1], 1.0)

    # ---- input DMAs: x 4-way across the two HWDGE queues (each queue
    # moves ~21GB/s, so chunking roughly halves the x latency); wqkv split
    # K/Q/V on the gpsimd SWDGE queue in consumption order. ----
    xtb = sb.tile([128, NT * 128], BF16, tag="xtb")
    nc.sync.dma_start(xtb[:, 0:512], xt_d[:, 0:512])
    nc.scalar.dma_start(xtb[:, 512:1024], xt_d[:, 512:1024])
    wqb = sb.tile([128, 3 * C], BF16, tag="wqb")
    nc.gpsimd.dma_start(wqb[:, C:2 * C], wqkv_d[:, C:2 * C])      # K first
    nc.gpsimd.dma_start(wqb[:, 0:C], wqkv_d[:, 0:C])              # Q
    vb2 = sb.tile([128, NT, HEADS, 2 * DH], BF16, tag="vb2")
    nc.gpsimd.dma_start(vb2[:], ones_bf_d[:])
    nc.gpsimd.dma_start(wqb[:, 2 * C:3 * C], wqkv_d[:, 2 * C:3 * C])  # V
    woa = const.tile([128, C], BF16, tag="woa")
    nc.sync.dma_start(woa[:], woa_d[:])
    wob = const.tile([128, C], BF16, tag="wob")
    nc.scalar.dma_start(wob[:], wob_d[:])
    xtb_flat = xtb[:]

    # ---- PE warm-up matmuls (overlap the x DMA; HAM clock-gate food) ----
    wmm_ps = stp.tile([128, 1024], FP32, tag="st")
    for w in range(N_WARM):
        nc.tensor.matmul(
            wmm_ps[:, (w % 2) * 512:(w % 2) * 512 + 512],
            lhsT=wmm_a[:, 0:128], rhs=wmm_a[:],
            start=True, stop=True, skip_group_check=True,
        )

    # ---- K^T, Q^T in x-chunk order (start as each x chunk lands) ----
    kt_ps = stp.tile([128, 1024], FP32, tag="st")
    for ih in range(2):
        nc.tensor.matmul(
            kt_ps[:, ih * 512:(ih + 1) * 512],
            lhsT=wqb[:, C:2 * C],
            rhs=xtb_flat[:, ih * 512:(ih + 1) * 512],
            start=True, stop=True,
        )
    qt_ps = stp.tile([128, 1024], FP32, tag="st")
    for ih in range(2):
        nc.tensor.matmul(
            qt_ps[:, ih * 512:(ih + 1) * 512],
            lhsT=wqb[:, 0:C],
            rhs=xtb_flat[:, ih * 512:(ih + 1) * 512],
            start=True, stop=True,
        )

    # ---- norms ----
    nsq = sb.tile([128, 2], FP32, tag="nsq")
    qsq_scr = sb.tile([128, 1024], FP32, tag="qsq_scr")
    nc.scalar.activation(qsq_scr[:], qt_ps[:], Square, accum_out=nsq[:, 0:1])
    # Q^T plain bf16 copy on ACT (no norm dependency)
    qtb = sb.tile([128, 1024], BF16, tag="qtb")
    nc.scalar.copy(qtb[:, 0:512], qt_ps[:, 0:512])
    nc.scalar.copy(qtb[:, 512:1024], qt_ps[:, 512:1024])
    # K^T bf16 copy on DVE (feeds the scaled ktbd blocks)
    ktb = sb.tile([128, 1024], BF16, tag="ktb")
    ktb_i = nc.vector.tensor_copy(ktb[:], kt_ps[:])
    ksq_scr = sb.tile([128, 1024], FP32, tag="ksq_scr")
    nc.scalar.activation(ksq_scr[:], ktb[:], Square, scale=0.1,
                         accum_out=nsq[:, 1:2])

    # V in [token, f] orientation, parked in the o_b banks (after the norm
    # inputs so the late wqb_v DMA doesn't block the K/Q path)
    v_ps = rps.tile([128, 1024], FP32, tag="ob")
    for t in range(NT):
        nc.tensor.matmul(
            v_ps[:, t * 128:(t + 1) * 128],
            lhsT=xtb_flat[:, t * 128:(t + 1) * 128],
            rhs=wqb[:, 2 * C:3 * C],
            start=True, stop=True,
        )
    # rsq via fp32 bit-hack + 1 Newton step: [:,0]=1/||q||, [:,1]=10/||k||
    nsqc = sb.tile([128, 2], FP32, tag="nsqc")
    nc.vector.tensor_scalar_max(nsqc[:], nsq[:], 1e-26)
    nni = nsqc[:].bitcast(mybir.dt.int32)
    yi = sb.tile([128, 2], mybir.dt.int32, tag="yi")
    shr_i = nc.vector.tensor_scalar(yi[:], nni, 1, None, op0=shr)
    nc.vector.tensor_scalar(yi[:], yi[:], -1, 0x5F3759DF, op0=mult, op1=add)
    y = yi[:].bitcast(FP32)
    nh = sb.tile([128, 2], FP32, tag="nh")
    nc.vector.tensor_scalar_mul(nh[:], nsqc[:], 0.5)
    t1 = sb.tile([128, 2], FP32, tag="t1")
    nc.vector.tensor_mul(t1[:], y, y)
    nc.vector.tensor_mul(t1[:], t1[:], nh[:])
    nwt2 = nc.vector.tensor_scalar(t1[:], t1[:], -1.0, 1.5, op0=mult, op1=add)
    nc.vector.tensor_mul(y, y, t1[:])
    rsq = y

    # dummy matmuls bridge the norm-chain latency so the HAM clock-gate
    # stays warm into the S pipeline; anchors spread them across the window.
    # The o_a banks are cleared later by PV(0,0)'s start=True.
    ndum = ops.tile([128, 1024], FP32, tag="oa")
    for w in range(N_NORMWARM):
        di = nc.tensor.matmul(
            ndum[:, (w % 2) * 512:(w % 2) * 512 + 512],
            lhsT=wmm_a[:, 0:128], rhs=wmm_a[:],
            start=True, stop=True, skip_group_check=True,
        )
        anchor = (ktb_i, ktb_i, shr_i, shr_i, shr_i, shr_i,
                  nwt2, nwt2)[w % 8]
        tile.add_dep_helper(di.ins, anchor.ins, reason="hold in norm window")

    # scaled block-diagonal K tiles via the masked per-partition scale
    s1 = sb.tile([128, 1], FP32, tag="s1")
    nc.vector.tensor_scalar(s1[:], rsq[:, 0:1], rsq[:, 1:2], None, op0=mult)
    m4 = sb.tile([128, 4], FP32, tag="m4")
    nc.vector.tensor_scalar(m4[:], mask4[:], s1[:, 0:1], None, op0=mult)

    def emit_ktbd(h):
        nc.vector.tensor_scalar(
            ktbd[:, h, :], ktb[:], m4[:, h:h + 1], None, op0=mult)
    emit_ktbd(0)
    emit_ktbd(1)

    # V scatter: vb2 slot cols [0:16]=V[:,0:16], [32:48]=V[:,16:32]
    # (ones at 16:32 and 48:64 ride in from the host background).
    v_src = v_ps[:].rearrange("p (t h s x) -> p t h s x", t=NT, h=HEADS, s=2)
    vb2_v = vb2[:].rearrange("p t h (s x) -> p t h s x", s=4)
    nc.vector.tensor_copy(vb2_v[:, :, :, 0, :], v_src[:, :, :, 0, :])
    nc.vector.tensor_copy(vb2_v[:, :, :, 2, :], v_src[:, :, :, 1, :])
    emit_ktbd(2)
    emit_ktbd(3)

    # ---- attention ----
    eb = sb.tile([128, NT, HEADS, 1024], BF16, tag="eb")
    o_a = ops.tile([128, 1024], FP32, tag="oa")  # [O0|r0 interleaved, O1|r1]
    o_b = rps.tile([128, 1024], FP32, tag="ob")  # heads 2,3

    def emit_s_exp(jt, h, st_pool_tag):
        if st_pool_tag == "oa":
            st = ops.tile([128, 1024], FP32, tag="oa", name=f"st_{jt}_{h}")
        elif st_pool_tag == "ob":
            st = rps.tile([128, 1024], FP32, tag="ob", name=f"st_{jt}_{h}")
        else:
            st = stp.tile([128, 1024], FP32, tag="st", name=f"st_{jt}_{h}")
        for ih in range(2):
            nc.tensor.matmul(
                st[:, ih * 512:(ih + 1) * 512],
                lhsT=ktbd[:, h, jt * 128:(jt + 1) * 128],
                rhs=qtb[:, ih * 512:(ih + 1) * 512],
                start=True, stop=True,
            )
        if (jt, h) in DVE_TILES:
            ebi = eb[:, jt, h, :].bitcast(I16)
            nc.vector.tensor_scalar(ebi, st[:], EXP_A, 16256.0 - EXP_C,
                                    op0=mult, op1=add)
        else:
            nc.scalar.activation(eb[:, jt, h, :], st[:], Exp)

    def emit_pv_pair(jt, heads):
        dst = o_a if heads[0] < 2 else o_b
        for ih in range(2):
            for h in heads:
                nc.tensor.matmul(
                    dst[64 * (h % 2):64 * (h % 2) + 64,
                        ih * 512:(ih + 1) * 512],
                    lhsT=vb2[:, jt, h, :],
                    rhs=eb[:, jt, h, ih * 512:(ih + 1) * 512],
                    start=(jt == 0), stop=(jt == NT - 1),
                    tile_position=(0, 64 * (h % 2)),
                    skip_group_check=True,
                )

    def normalize(o_acc, stack, tag):
        r = sb.tile([128, 1024], FP32, tag=f"r_{tag}")
        ri = nc.vector.reciprocal_approx_fast(r[:], o_acc[:])
        rs = sb.tile([128, 1024], FP32, tag=f"rs_{tag}")
        nc.vector.stream_shuffle(rs[:], r[:], SHUF_MASK)
        nc.vector.tensor_mul(stack[:], o_acc[:], rs[:])
        return ri

    stack_a = sb.tile([128, 1024], BF16, tag="stack_a")
    stack_b = sb.tile([128, 1024], BF16, tag="stack_b")

    # ---- phase A: heads 0,1 (S buffers: stp x2 + the o_b banks) ----
    tags_a = ["st", "st", "ob"]
    n = 0
    for jt in range(NT):
        for h in (0, 1):
            emit_s_exp(jt, h, tags_a[n % 3])
            n += 1
        if jt >= 2:
            emit_pv_pair(jt - 2, (0, 1))
    emit_pv_pair(NT - 2, (0, 1))
    emit_pv_pair(NT - 1, (0, 1))

    # phase-A tail (overlaps phase B): normalize h01; the projection runs at
    # the end so the retired o_a banks serve as extra phase-B S buffers.
    normalize(o_a, stack_a, "a")

    # ---- phase B: heads 2,3 (S buffers: stp x2, + the o_a banks once the
    # phase-A normalize has consumed them) ----
    tags_b = ["st"] * 16
    for i in (7, 10, 13):
        tags_b[i] = "oa"
    n = 0
    for jt in range(NT):
        for h in (2, 3):
            emit_s_exp(jt, h, tags_b[n])
            n += 1
        if jt >= 2:
            emit_pv_pair(jt - 2, (2, 3))
    emit_pv_pair(NT - 2, (2, 3))
    emit_pv_pair(NT - 1, (2, 3))

    rbi = normalize(o_b, stack_b, "b")
    # y^T accumulates where o_a lived; proj_a starts each bank, proj_b stops.
    y_ps = ops.tile([128, 1024], FP32, tag="oa", name="y_ps")
    for ih in range(2):
        nc.tensor.matmul(
            y_ps[:, ih * 512:(ih + 1) * 512],
            lhsT=woa[:],
            rhs=stack_a[:, ih * 512:(ih + 1) * 512],
            start=True, stop=False,
            skip_group_check=True,
        )
    # keep the PE warm through the h23 normalize chain (anchored so the
    # scheduler can't hoist them into the pipeline).
    tdum = stp.tile([128, 1024], FP32, tag="st")
    for w in range(N_TAILWARM):
        nc.tensor.matmul(
            tdum[:, (w % 2) * 512:(w % 2) * 512 + 512],
            lhsT=wmm_a[:, 0:128], rhs=wmm_a[:],
            start=True, stop=True, skip_group_check=True,
        )
    for ih in range(2):
        nc.tensor.matmul(
            y_ps[:, ih * 512:(ih + 1) * 512],
            lhsT=wob[:],
            rhs=stack_b[:, ih * 512:(ih + 1) * 512],
            start=False, stop=True,
            skip_group_check=True,
        )
    yout = sb.tile([128, 1024], BF16, tag="yout")
    nc.scalar.copy(yout[:, 0:512], y_ps[:, 0:512])
    nc.vector.tensor_copy(yout[:, 512:1024], y_ps[:, 512:1024])
    nc.sync.dma_start(out_d[0:64, :], yout[0:64, :])
    nc.scalar.dma_start(out_d[64:128, :], yout[64:128, :])


def build_nc():
    nc = bacc.Bacc("TRN2", target_bir_lowering=False, debug=False,
                   num_devices=N_CORES)
    xt_d = nc.dram_tensor("xt", [128, HW], BF16, kind="ExternalInput").ap()
    wqkv_d = nc.dram_tensor("w_qkv_bf", [C, 3 * C], BF16, kind="ExternalInput").ap()
    woa_d = nc.dram_tensor("woa", [128, C], BF16, kind="ExternalInput").ap()
    wob_d = nc.dram_tensor("wob", [128, C], BF16, kind="ExternalInput").ap()
    ones_bf_d = nc.dram_tensor("ones_bf", [128, NT, HEADS, 2 * DH], BF16,
                               kind="ExternalInput").ap()
    # transposed output: y^T [c, i'] with i' = t*128 + p <-> token 8p+t
    out_d = nc.dram_tensor("out", [C, HW], BF16, kind="ExternalOutput").ap()
    with tile.TileContext(nc) as tc:
        with ExitStack() as ctx:
            build_kernel_body(ctx, tc, out_d, xt_d, wqkv_d,
                              woa_d, wob_d, ones_bf_d)
    nc.compile()
    return nc


_CACHED_NC = None


def get_nc():
    global _CACHED_NC
    if _CACHED_NC is None:
        _CACHED_NC = build_nc()
    return _CACHED_NC


def _interleave_wout_rows(w_half):
    """w_half: [64, C] (two heads' d rows).  Rows for the 16-interleaved
    stack layout: [h0 d0:16; 0; h0 d16:32; 0; h1 d0:16; 0; h1 d16:32; 0]."""
    out = np.zeros((128, C), dtype=np.float32)
    out[0:16] = w_half[0:16]
    out[32:48] = w_half[16:32]
    out[64:80] = w_half[32:48]
    out[96:112] = w_half[48:64]
    return out


def make_in_maps(x, w_qkv, w_out, b_out):
    x = np.ascontiguousarray(np.asarray(x, dtype=np.float32)).reshape(N_CORES, HW, C)
    xt = np.ascontiguousarray(
        x.reshape(N_CORES, 128, NT, C).transpose(0, 3, 2, 1).reshape(N_CORES, C, HW)
    ).astype(ml_dtypes.bfloat16)
    w_qkv_bf = np.asarray(w_qkv, dtype=np.float32).astype(ml_dtypes.bfloat16)
    w_out = np.asarray(w_out, dtype=np.float32)

    woa = _interleave_wout_rows(w_out[0:64]).astype(ml_dtypes.bfloat16)
    wob = _interleave_wout_rows(w_out[64:128]).astype(ml_dtypes.bfloat16)
    # vb2 background: ones in the 16-col denominator slots (s=1 and s=3)
    ones_bf = np.zeros((128, NT, HEADS, 2 * DH), dtype=ml_dtypes.bfloat16)
    v4 = ones_bf.reshape(128, NT, HEADS, 4, 16)
    v4[:, :, :, 1, :] = 1.0
    v4[:, :, :, 3, :] = 1.0
    return [
        {"xt": xt[i], "w_qkv_bf": w_qkv_bf, "woa": woa, "wob": wob,
         "ones_bf": ones_bf}
        for i in range(N_CORES)
    ]


def kernel(x, w_qkv, w_out, b_out, _trace=False, _trace_kwargs=None):
    nc = get_nc()
    in_maps = make_in_maps(x, w_qkv, w_out, b_out)
    res = run_bass_kernel_spmd(
        nc, in_maps, core_ids=list(range(N_CORES)),
        trace=_trace, **(_trace_kwargs or {}),
    )
    b_out_f = np.asarray(b_out, dtype=np.float32).reshape(C)
    outs = []
    for i in range(N_CORES):
        yt = np.asarray(res.results[i]["out"]).astype(np.float32)
        y = yt.reshape(C, NT, 128).transpose(2, 1, 0).reshape(HW, C)
        outs.append(y + b_out_f[None, :])
    out = np.stack(outs).reshape(8, 32, 32, 128).astype(np.float32)
    if _trace:
        kernel.last_result = res
    return out
